# revision 21
# baseline (speedup 1.0000x reference)
"""Causal self-attention (B=4, T=2048, C=1024, H=16, D=64) on 8 TRN2 cores.

Sharding: 4-way data parallel on batch x 2-way tensor parallel on heads.
Core c handles batch b = c // 2 and heads (c % 2) * 8 .. (c % 2) * 8 + 7.
Each core computes a partial projection output [T, C]; the host sums the
two partials per batch and adds proj_b.

All transposes and bf16 casts are done host-side; the device consumes:
  xT   [C, T]  bf16    x[b].T
  wqkT [C, 1024] bf16  qkv_w rows for this core's q then k heads, transposed
  wvT  [C, 512] bf16   qkv_w rows for this core's v heads, transposed
  bqk  [1024] f32      qkv_b slice (q rows then k rows)
  bv   [512] f32       qkv_b slice for v rows
  pwT  [512, C] bf16   proj_w[:, this core's head columns].T
and produces  part [T, C] f32  (partial projection output, pre-bias).

Device dataflow per core (all matmul inputs bf16, PSUM accumulation f32):
  V-gen:  V[t, o] = xT.T @ wvT, stored per head as V_ext [128, T/128, 65]
          bf16 with a ones column (index 64) that accumulates softmax
          denominators during the PV matmul.
  per head-pair p (heads 2p, 2p+1 packed 64+64 on partitions):
    QK-gen: Q^T/K^T [o, t] chunks via wqkT.T @ xT (+bias on evacuation).
    per q-tile J (512 queries):
      S^T[k, q] chunks (K=64 row-packed pairs, both heads into one
      2-bank PSUM tile), one fused exp over both banks -> P^T bf16
      (no max subtraction - inputs are N(0,1)-scale so logits are
      small), causal mask applied in-place by gpsimd affine_select on
      diagonal chunks, then O_ext^T[65, q] += V_ext.T @ P^T over k.
      Row 64 of O_ext^T is the denominator: the PSUM tile is copied to
      SBUF, the denominator row is bounced through DRAM to broadcast
      across partitions, reciprocal'd, and multiplied in during the
      store to O^T (bf16).
  proj:  partial[t, o] = O^T.T @ pwT, f32 out, streamed to DRAM.
  Emission order (V-gen, then per-pair QK-gen + attention, proj last)
  lets Tile overlap pair p+1's QK matmuls with pair p's exp-bound
  attention, and the projection with the last pair's attention.
"""

import ml_dtypes
import numpy as np

B, T, C = 4, 2048, 1024
H, D = 16, 64
HPC = 8          # heads per core
OC = HPC * D     # 512 rows for each of q, k, v per core
NCORES = 8

TRACE = False          # set by test harness to capture a profile
LAST_RESULT = None     # BassKernelResults of the most recent run


def _build(T_=T):
    import contextlib

    import concourse.bass as bass
    import concourse.mybir as mybir
    import concourse.tile as tile
    from concourse import bacc

    f32 = mybir.dt.float32
    bf16 = mybir.dt.bfloat16
    Act = mybir.ActivationFunctionType

    NTT = T_ // 512      # 512-wide t tiles
    NKC = T_ // 128      # 128-wide k chunks
    NJ = T_ // 512       # q tiles

    nc = bacc.Bacc(trn_type="TRN2")

    xT = nc.dram_tensor("xT", [C, T_], bf16, kind="ExternalInput")
    wqkT = nc.dram_tensor("wqkT", [C, 2 * OC], bf16, kind="ExternalInput")
    wvT = nc.dram_tensor("wvT", [C, OC], bf16, kind="ExternalInput")
    bqk = nc.dram_tensor("bqk", [2 * OC], f32, kind="ExternalInput")
    bv = nc.dram_tensor("bv", [OC], f32, kind="ExternalInput")
    pwT = nc.dram_tensor("pwT", [OC, C], bf16, kind="ExternalInput")
    part = nc.dram_tensor("part", [T_, C], f32, kind="ExternalOutput")

    with tile.TileContext(nc) as tc:
        ctx = contextlib.ExitStack()
        with ctx:
            singles = ctx.enter_context(tc.tile_pool(name="singles", bufs=1))
            xpool = ctx.enter_context(tc.tile_pool(name="xpool", bufs=NTT))
            ptpool = ctx.enter_context(tc.tile_pool(name="ptpool", bufs=6))
            otfpool = ctx.enter_context(tc.tile_pool(name="otfpool", bufs=3))
            bcpool = ctx.enter_context(tc.tile_pool(name="bcpool", bufs=3))
            outpool = ctx.enter_context(tc.tile_pool(name="outpool", bufs=3))
            drampool = ctx.enter_context(
                tc.tile_pool(name="drampool", bufs=4, space="DRAM")
            )
            pspool = ctx.enter_context(
                tc.tile_pool(name="pspool", bufs=3, space="PSUM")
            )
            ps_ot = ctx.enter_context(
                tc.tile_pool(name="ps_ot", bufs=2, space="PSUM")
            )

            # ---- inputs (biases + wv + first x tile first) ----
            bqk_sb = singles.tile([128, 8], f32)
            nc.sync.dma_start(bqk_sb[:, :], bqk[:].rearrange("(j p) -> p j", p=128))
            bv_sb = singles.tile([128, OC], f32)
            nc.sync.dma_start(
                bv_sb[:, :], bv[:].unsqueeze(0).partition_broadcast(128)
            )
            wv_sb = singles.tile([128, 8, OC], bf16)
            for cc in range(8):
                nc.sync.dma_start(
                    wv_sb[:, cc, :], wvT[cc * 128 : (cc + 1) * 128, :]
                )
            xts = []
            for tt in range(NTT):
                xt = xpool.tile([128, 8, 512], bf16, tag="xt", name=f"xt{tt}")
                nc.sync.dma_start(
                    xt[:, :, :],
                    xT[:, tt * 512 : (tt + 1) * 512].rearrange(
                        "(cc p) t -> p cc t", p=128
                    ),
                )
                xts.append(xt)
                if tt == 0:
                    wqk_sb = singles.tile([128, 8, 2 * OC], bf16)
                    for cc in range(8):
                        nc.sync.dma_start(
                            wqk_sb[:, cc, :], wqkT[cc * 128 : (cc + 1) * 128, :]
                        )
            pw_sb = singles.tile([128, 4, C], bf16)
            for cc in range(4):
                nc.sync.dma_start(
                    pw_sb[:, cc, :], pwT[cc * 128 : (cc + 1) * 128, :]
                )

            # persistent activations; ones memset goes first so the PE
            # warm-up matmuls are not queued behind the big vext memset
            ones_sb = singles.tile([128, 64], bf16)
            nc.vector.memset(ones_sb[:, :], 1.0)

            # keep the PE busy (and HAM un-throttled) while inputs stream in
            warm = pspool.tile([128, 2, 512], f32, tag="ps", name="warm")
            for _ in range(130):
                nc.tensor.matmul(
                    warm[0:64, 0, 0:64],
                    ones_sb[0:64, :],
                    ones_sb[0:64, :],
                    start=True,
                    stop=True,
                )

            qkT = singles.tile([128, 8, T_], bf16)  # 4 q-pair + 4 k-pair chunks
            vext = singles.tile([128, HPC, NKC, 65], bf16)
            nc.vector.memset(vext[:, :, :, :], 1.0)
            otstore = singles.tile([128, 4, T_], bf16)

            def qk_group(j, tt):
                ps = pspool.tile([128, 2, 512], f32, tag="ps", name=f"qk{j}{tt}")
                for cc in range(8):
                    nc.tensor.matmul(
                        ps[:, 0, :],
                        wqk_sb[:, cc, j * 128 : (j + 1) * 128],
                        xts[tt][:, cc, :],
                        start=(cc == 0),
                        stop=(cc == 7),
                    )
                nc.vector.tensor_scalar_add(
                    qkT[:, j, tt * 512 : (tt + 1) * 512],
                    ps[:, 0, :],
                    bqk_sb[:, j : j + 1],
                )

            def proj_group(tch, oo):
                ps = pspool.tile([128, 2, 512], f32, tag="ps", name=f"pr{tch}{oo}")
                for cc in range(4):
                    nc.tensor.matmul(
                        ps[:, 0, :],
                        otstore[:, cc, tch * 128 : (tch + 1) * 128],
                        pw_sb[:, cc, oo * 512 : (oo + 1) * 512],
                        start=(cc == 0),
                        stop=(cc == 3),
                    )
                osb = outpool.tile([128, 512], f32, tag="osb", name=f"ob{tch}{oo}")
                nc.vector.tensor_copy(osb[:, :], ps[:, 0, :])
                nc.sync.dma_start(
                    part[tch * 128 : (tch + 1) * 128, oo * 512 : (oo + 1) * 512],
                    osb[:, :],
                )

            def attention_unit(p, J, filler=None):
                nkc = 4 * J + 4
                qsl = slice(J * 512, (J + 1) * 512)
                otp = [
                    ps_ot.tile([65, 512], f32, tag="ot", name=f"ot{p}{J}{h2}")
                    for h2 in range(2)
                ]
                sts = {}

                def flush(kc):
                    st2 = sts.pop(kc)
                    pt2 = ptpool.tile(
                        [128, 2, 512], bf16, tag="pt", name=f"pt{p}{J}{kc}"
                    )
                    nc.scalar.activation(
                        pt2[:, :, :], st2[:, :, :], Act.Exp, scale=0.125
                    )
                    m = kc - 4 * J
                    for h2 in range(2):
                        if m >= 0:
                            nc.gpsimd.affine_select(
                                out=pt2[:, h2, :],
                                in_=pt2[:, h2, :],
                                compare_op=mybir.AluOpType.is_ge,
                                fill=0.0,
                                base=-128 * m,
                                pattern=[[1, 512]],
                                channel_multiplier=-1,
                            )
                        nc.tensor.matmul(
                            otp[h2][:, :],
                            vext[:, 2 * p + h2, kc, :],
                            pt2[:, h2, :],
                            start=(kc == 0),
                            stop=(kc == nkc - 1),
                        )

                for kc in range(nkc):
                    st2 = pspool.tile(
                        [128, 2, 512], f32, tag="ps", name=f"st{p}{J}{kc}"
                    )
                    for h2 in range(2):
                        lo = 64 * h2
                        nc.tensor.matmul(
                            st2[:, h2, :],
                            qkT[lo : lo + 64, 4 + p, kc * 128 : (kc + 1) * 128],
                            qkT[lo : lo + 64, p, qsl],
                            start=True,
                            stop=True,
                        )
                    sts[kc] = st2
                    if kc >= 2:
                        flush(kc - 2)
                flush(nkc - 2)
                flush(nkc - 1)
                if filler is not None:
                    filler()

                # normalize + evacuate: PSUM -> SBUF copy, then broadcast the
                # denominator row across partitions with a ones matmul,
                # reciprocal, and scale while storing to O^T
                for h2 in range(2):
                    otf = otfpool.tile(
                        [65, 512], f32, tag="otf", name=f"of{p}{J}{h2}"
                    )
                    nc.vector.tensor_copy(otf[:, :], otp[h2][:, :])
                    drb = otfpool.tile([65, 512], bf16, tag="drb", name=f"dr{p}{J}{h2}")
                    nc.vector.tensor_copy(drb[64:65, :], otp[h2][64:65, :])
                    den = ps_ot.tile([64, 512], f32, tag="ot", name=f"dn{p}{J}{h2}")
                    nc.tensor.matmul(
                        den[:, :],
                        ones_sb[64:65, :],
                        drb[64:65, :],
                        start=True,
                        stop=True,
                    )
                    bc = bcpool.tile(
                        [64, 512], f32, tag="bc", name=f"bc{p}{J}{h2}"
                    )
                    nc.vector.reciprocal_approx_fast(out=bc[:, :], in_=den[:, :])
                    nc.vector.tensor_mul(
                        otstore[64 * h2 : 64 * h2 + 64, p, qsl],
                        otf[0:64, :],
                        bc[:, :],
                    )

            # ---- V generation + pair-0 QK generation + pair-0 attention,
            # per t-tile so compute starts as soon as the first x tile and
            # weights land ----
            for tt in range(NTT):
                for ts_ in range(4):
                    ps = pspool.tile([128, 2, 512], f32, tag="ps", name=f"v{tt}{ts_}")
                    for cc in range(8):
                        nc.tensor.matmul(
                            ps[:, 0, :],
                            xts[tt][:, cc, ts_ * 128 : (ts_ + 1) * 128],
                            wv_sb[:, cc, :],
                            start=(cc == 0),
                            stop=(cc == 7),
                        )
                    kc = tt * 4 + ts_
                    nc.vector.tensor_add(
                        vext[:, :, kc, 0:64],
                        ps[:, 0, :].rearrange("p (h e) -> p h e", h=HPC),
                        bv_sb[:, :].rearrange("p (h e) -> p h e", h=HPC),
                    )
                qk_group(0, tt)
                qk_group(4, tt)
                attention_unit(
                    0, tt, filler=lambda tt=tt: (qk_group(1, tt), qk_group(5, tt))
                )

            # ---- pairs 1-3: attention with next pair's QK generation and,
            # for the last pair, the projection interleaved (one unit behind
            # so the normalize chain has drained) ----
            for p in range(1, 4):
                next_groups = (
                    [(p + 1, tt) for tt in range(NTT)]
                    + [(5 + p, tt) for tt in range(NTT)]
                    if p < 3
                    else []
                )
                gper = len(next_groups) // NJ
                for J in range(NJ):
                    if p < 3:
                        def filler(J=J, gper=gper, ng=next_groups):
                            for j_, tt_ in ng[J * gper : (J + 1) * gper]:
                                qk_group(j_, tt_)
                    elif J > 0:
                        def filler(J=J):
                            for tch in range(4 * (J - 1), 4 * J):
                                proj_group(tch, 0)
                                proj_group(tch, 1)
                    else:
                        filler = None
                    attention_unit(p, J, filler=filler)

            # ---- projection tail (last q-tile) ----
            for tch in range(4 * (NJ - 1), T_ // 128):
                proj_group(tch, 0)
                proj_group(tch, 1)

    nc.compile()
    return nc


def make_in_maps(x, qkv_w, qkv_b, proj_w):
    """Shard full inputs into the 8 per-core input maps."""
    x = np.asarray(x, dtype=np.float32)
    qkv_w = np.asarray(qkv_w, dtype=np.float32)
    qkv_b = np.asarray(qkv_b, dtype=np.float32)
    proj_w = np.asarray(proj_w, dtype=np.float32)
    bf = ml_dtypes.bfloat16
    in_maps = []
    for c in range(NCORES):
        b, g = divmod(c, 2)
        hs = np.arange(g * HPC, (g + 1) * HPC)
        rows = (hs[:, None] * D + np.arange(D)[None, :]).ravel()
        qk_rows = np.concatenate([rows, C + rows])
        v_rows = 2 * C + rows
        in_maps.append(
            {
                "xT": np.ascontiguousarray(x[b].T).astype(bf),
                "wqkT": np.ascontiguousarray(qkv_w[qk_rows].T).astype(bf),
                "wvT": np.ascontiguousarray(qkv_w[v_rows].T).astype(bf),
                "bqk": np.ascontiguousarray(qkv_b[qk_rows]),
                "bv": np.ascontiguousarray(qkv_b[v_rows]),
                "pwT": np.ascontiguousarray(proj_w[:, rows].T).astype(bf),
            }
        )
    return in_maps


def kernel(x, qkv_w, qkv_b, proj_w, proj_b):
    global LAST_RESULT
    from concourse.bass_utils import run_bass_kernel_spmd

    nc = _build(T)
    in_maps = make_in_maps(x, qkv_w, qkv_b, proj_w)
    res = run_bass_kernel_spmd(nc, in_maps, list(range(NCORES)), trace=TRACE)
    LAST_RESULT = res
    proj_b = np.asarray(proj_b, dtype=np.float32)
    out = np.empty((B, T, C), dtype=np.float32)
    for b in range(B):
        out[b] = res.results[2 * b]["part"] + res.results[2 * b + 1]["part"]
        out[b] += proj_b[None, :]
    return out
